# revision 17
# baseline (speedup 1.0000x reference)
"""Clipped shallow PLRNN recurrence on 8 TRN2 NeuronCores (Bass/Tile).

z_{t+1} = A*z_t + (relu(z_t@W2 + h2) - relu(z_t@W2)) @ W1 + h1
x_t     = z_t @ OB + Ob            -> output [bs, nt+1, ns]

Strategy (data-parallel over batch, 32 lanes/core; sequential scan local):
 - state kept transposed zT [65, B] (64 states + constant ones row)
 - relu(y+h2)-relu(y) == |h2| * clamp(sigma*y/|h2|, -1, 0) + relu(h2):
     W2h = W2 * (sign(h2)/|h2|)  (host precompute)    -> mm1 yields yhat
     g   = min(max(yhat, -1), 0)                      (one DVE op, immediates)
     W1h = W1 * |h2|                                  -> mm2 accumulates g@W1h
     h1p = h1 + relu(h2)@W1  folded into the Az matmul bias row
 - A*z + h1p via one matmul with lhsT = [diag(A); h1p] [65,64], rhs = zT
   (ones row of zT supplies the bias)
 - x_t = z_t@OB + Ob via lhsT = zT [65,B], rhs = [OB; Ob] [65,64]
 - per-core batch split into G groups whose chains interleave on the engines
"""

import sys

sys.path.insert(0, "/opt/trn_rl_repo")

import numpy as np

NS = 64      # n_states
NH = 256     # n_hidden
BS = 256     # batch
NCORES = 8
NT = 4096
B = BS // NCORES          # 32 lanes per core

# tunables
G = 2                     # pipelined batch groups per core
U = 64                    # time steps per For_i iteration (DMA chunk)
OBS = 8                   # obs psum slots per bank (batched psum->sbuf copy)

_F32 = None  # set lazily


def _build_program(nt, u, g_groups, nt_run=None, timing_mode=False,
                   staggered_reset=False, obs_batch=False, no_xdma=False,
                   xdma_engine="sync", xsb_bufs=2):
    import concourse.bacc as bacc
    import concourse.mybir as mybir
    import concourse.tile as tile
    from concourse.bass import ds

    f32 = mybir.dt.float32
    bg = B // g_groups
    if nt_run is None:
        nt_run = nt
    assert nt % u == 0 and nt_run % u == 0 and u % OBS == 0 and u % 2 == 0
    if obs_batch:
        return _build_program_obsbatch(
            nt, u, g_groups, nt_run, timing_mode, staggered_reset
        )

    nc = bacc.Bacc("TRN2", target_bir_lowering=False, debug=False)

    z0t = nc.dram_tensor("z0t", [NS + 1, B], f32, kind="ExternalInput")
    w2h = nc.dram_tensor("w2h", [NS, NH], f32, kind="ExternalInput")
    w1h = nc.dram_tensor("w1h", [NH, NS], f32, kind="ExternalInput")
    azm = nc.dram_tensor("azm", [NS + 1, NS], f32, kind="ExternalInput")
    obb = nc.dram_tensor("obb", [NS + 1, NS], f32, kind="ExternalInput")
    x = nc.dram_tensor("x", [B, nt + 1, NS], f32, kind="ExternalOutput")

    Copy = mybir.ActivationFunctionType.Copy
    AOp = mybir.AluOpType

    with tile.TileContext(nc) as tc:
        from contextlib import ExitStack

        with ExitStack() as ctx:
            const = ctx.enter_context(tc.tile_pool(name="const", bufs=1))
            state = ctx.enter_context(tc.tile_pool(name="state", bufs=1))
            gpool = ctx.enter_context(tc.tile_pool(name="gp", bufs=1))
            xsb = ctx.enter_context(tc.tile_pool(name="xsb", bufs=xsb_bufs))
            ypsum = ctx.enter_context(tc.tile_pool(name="yps", bufs=1, space="PSUM"))
            zpsum = ctx.enter_context(tc.tile_pool(name="zps", bufs=1, space="PSUM"))
            xpsum = ctx.enter_context(tc.tile_pool(name="xps", bufs=2, space="PSUM"))

            w2sb = const.tile([NS, NH], f32, tag="w2")
            w1sb = const.tile([128, 2, NS], f32, tag="w1")
            azsb = const.tile([NS + 1, NS], f32, tag="az")
            obsb = const.tile([NS + 1, NS], f32, tag="ob")
            nc.sync.dma_start(out=w2sb, in_=w2h[:, :])
            nc.sync.dma_start(out=w1sb, in_=w1h.rearrange("(c p) m -> p c m", c=2))
            nc.sync.dma_start(out=azsb, in_=azm[:, :])
            nc.sync.dma_start(out=obsb, in_=obb[:, :])

            # per-group ping-pong state tiles [65, bg]
            zs = []
            for g in range(g_groups):
                za = state.tile([NS + 1, bg], f32, tag=f"zA{g}")
                zb = state.tile([NS + 1, bg], f32, tag=f"zB{g}")
                nc.sync.dma_start(out=za, in_=z0t[:, g * bg:(g + 1) * bg])
                nc.vector.memset(zb[NS:NS + 1, :], 1.0)
                zs.append((za, zb))

            # t = 0 observation (per group; engine APs must sit at partition 0)
            for g in range(g_groups):
                px0 = xpsum.tile([bg, 1, NS], f32, tag=f"xp{g}", name=f"px0{g}")
                nc.tensor.matmul(
                    px0[:, 0, :], lhsT=zs[g][0], rhs=obsb, start=True, stop=True
                )
                x0sb = xsb.tile([bg, 1, NS], f32, tag=f"xt{g}", name=f"x0sb{g}")
                nc.scalar.activation(out=x0sb, in_=px0, func=Copy)
                nc.sync.dma_start(
                    out=x[g * bg:(g + 1) * bg, 0:1, :], in_=x0sb
                )

            with tc.For_i(
                0, nt_run, u, hint_engines=(mybir.EngineType.PE,),
                staggered_reset=staggered_reset,
            ) as it:
                xts = [
                    xsb.tile([bg, u, NS], f32, tag=f"xt{g}", name=f"xt{g}")
                    for g in range(g_groups)
                ]
                pxs = [None] * g_groups
                for k in range(u):
                    for g in range(g_groups):
                        za, zb = zs[g]
                        src, dst = (za, zb) if k % 2 == 0 else (zb, za)
                        # mm1: yhat^T chunks [128, 2, bg]
                        py = ypsum.tile([128, 2, bg], f32, tag=f"y{g}")
                        nc.tensor.matmul(
                            py[:, 0, :], lhsT=w2sb[:, 0:128], rhs=src[0:NS, :],
                            start=True, stop=True,
                        )
                        nc.tensor.matmul(
                            py[:, 1, :], lhsT=w2sb[:, 128:256], rhs=src[0:NS, :],
                            start=True, stop=True,
                        )
                        # clamp to [-1, 0]
                        gt = gpool.tile([128, 2, bg], f32, tag=f"g{g}")
                        nc.vector.tensor_scalar(
                            out=gt, in0=py,
                            scalar1=-1.0, scalar2=0.0,
                            op0=AOp.max, op1=AOp.min,
                        )
                        # z_{t+1} = A*z + h1p + g@W1h   (accumulated in psum)
                        pz = zpsum.tile([NS, bg], f32, tag=f"z{g}")
                        nc.tensor.matmul(pz, lhsT=azsb, rhs=src, start=True, stop=False)
                        nc.tensor.matmul(
                            pz, lhsT=w1sb[:, 0, :], rhs=gt[:, 0, :],
                            start=False, stop=False,
                        )
                        nc.tensor.matmul(
                            pz, lhsT=w1sb[:, 1, :], rhs=gt[:, 1, :],
                            start=False, stop=True,
                        )
                        nc.scalar.activation(out=dst[0:NS, :], in_=pz, func=Copy)
                        # observation of z_{t+1}
                        if k % OBS == 0:
                            pxs[g] = xpsum.tile(
                                [bg, OBS, NS], f32, tag=f"xp{g}", name=f"px{g}"
                            )
                        nc.tensor.matmul(
                            pxs[g][:, k % OBS, :], lhsT=dst, rhs=obsb,
                            start=True, stop=True,
                        )
                        if k % OBS == OBS - 1:
                            nc.scalar.activation(
                                out=xts[g][:, k - (OBS - 1):k + 1, :],
                                in_=pxs[g], func=Copy,
                            )
                for g in range(g_groups):
                    if no_xdma:
                        continue
                    dest_t = ds(1, u) if timing_mode else ds(it + 1, u)
                    eng = nc.gpsimd if xdma_engine == "gpsimd" else nc.sync
                    eng.dma_start(
                        out=x[g * bg:(g + 1) * bg, dest_t, :], in_=xts[g]
                    )

    nc.compile()
    return nc


def _build_program_obsbatch(nt, u, g_groups, nt_run, timing_mode,
                            staggered_reset):
    """Variant: z history kept in an 8-slot rotating buffer per group;
    one [65,8*bg]x[65,64] obs matmul per 8 steps replaces 8 small ones."""
    import concourse.bacc as bacc
    import concourse.mybir as mybir
    import concourse.tile as tile
    from concourse.bass import ds
    from contextlib import ExitStack

    f32 = mybir.dt.float32
    bg = B // g_groups
    S = OBS  # history slots
    assert u % S == 0

    nc = bacc.Bacc("TRN2", target_bir_lowering=False, debug=False)

    z0t = nc.dram_tensor("z0t", [NS + 1, B], f32, kind="ExternalInput")
    w2h = nc.dram_tensor("w2h", [NS, NH], f32, kind="ExternalInput")
    w1h = nc.dram_tensor("w1h", [NH, NS], f32, kind="ExternalInput")
    azm = nc.dram_tensor("azm", [NS + 1, NS], f32, kind="ExternalInput")
    obb = nc.dram_tensor("obb", [NS + 1, NS], f32, kind="ExternalInput")
    x = nc.dram_tensor("x", [B, nt + 1, NS], f32, kind="ExternalOutput")

    Copy = mybir.ActivationFunctionType.Copy
    AOp = mybir.AluOpType

    with tile.TileContext(nc) as tc:
        with ExitStack() as ctx:
            const = ctx.enter_context(tc.tile_pool(name="const", bufs=1))
            state = ctx.enter_context(tc.tile_pool(name="state", bufs=1))
            gpool = ctx.enter_context(tc.tile_pool(name="gp", bufs=1))
            xsb = ctx.enter_context(tc.tile_pool(name="xsb", bufs=xsb_bufs))
            ypsum = ctx.enter_context(tc.tile_pool(name="yps", bufs=1, space="PSUM"))
            zpsum = ctx.enter_context(tc.tile_pool(name="zps", bufs=1, space="PSUM"))
            xpsum = ctx.enter_context(tc.tile_pool(name="xps", bufs=2, space="PSUM"))

            w2sb = const.tile([NS, NH], f32, tag="w2")
            w1sb = const.tile([128, 2, NS], f32, tag="w1")
            azsb = const.tile([NS + 1, NS], f32, tag="az")
            obsb = const.tile([NS + 1, NS], f32, tag="ob")
            nc.sync.dma_start(out=w2sb, in_=w2h[:, :])
            nc.sync.dma_start(out=w1sb, in_=w1h.rearrange("(c p) m -> p c m", c=2))
            nc.sync.dma_start(out=azsb, in_=azm[:, :])
            nc.sync.dma_start(out=obsb, in_=obb[:, :])

            # per-group rotating z history [65, S, bg]; slot j holds z_{t}
            # with t % S == j. ones row constant across slots.
            zh = []
            for g in range(g_groups):
                zhg = state.tile([NS + 1, S, bg], f32, tag=f"zh{g}", name=f"zh{g}")
                nc.vector.memset(zhg[NS:NS + 1, :, :], 1.0)
                # z0 lives in slot S-1 (step k reads slot (k-1) % S)
                nc.sync.dma_start(
                    out=zhg[0:NS, S - 1, :], in_=z0t[0:NS, g * bg:(g + 1) * bg]
                )
                zh.append(zhg)

            # t = 0 observation from slot S-1
            for g in range(g_groups):
                px0 = xpsum.tile([bg, NS], f32, tag=f"xp{g}", name=f"px0{g}")
                nc.tensor.matmul(
                    px0, lhsT=zh[g][:, S - 1, :], rhs=obsb, start=True, stop=True
                )
                x0sb = xsb.tile([bg, NS], f32, tag=f"x0sb{g}", name=f"x0sb{g}")
                nc.scalar.activation(out=x0sb, in_=px0, func=Copy)
                nc.sync.dma_start(
                    out=x[g * bg:(g + 1) * bg, 0, :], in_=x0sb
                )

            with tc.For_i(
                0, nt_run, u, hint_engines=(mybir.EngineType.PE,),
                staggered_reset=staggered_reset,
            ) as it:
                xst = [
                    xsb.tile([S * bg, u // S, NS], f32, tag=f"xt{g}", name=f"xst{g}")
                    for g in range(g_groups)
                ]
                for k in range(u):
                    for g in range(g_groups):
                        zhg = zh[g]
                        src = zhg[:, (k - 1) % S, :]
                        dst = zhg[:, k % S, :]
                        py = ypsum.tile([128, 2, bg], f32, tag=f"y{g}")
                        nc.tensor.matmul(
                            py[:, 0, :], lhsT=w2sb[:, 0:128], rhs=src[0:NS, :],
                            start=True, stop=True,
                        )
                        nc.tensor.matmul(
                            py[:, 1, :], lhsT=w2sb[:, 128:256], rhs=src[0:NS, :],
                            start=True, stop=True,
                        )
                        gt = gpool.tile([128, 2, bg], f32, tag=f"g{g}")
                        nc.vector.tensor_scalar(
                            out=gt, in0=py,
                            scalar1=-1.0, scalar2=0.0,
                            op0=AOp.max, op1=AOp.min,
                        )
                        pz = zpsum.tile([NS, bg], f32, tag=f"z{g}")
                        nc.tensor.matmul(pz, lhsT=azsb, rhs=src, start=True, stop=False)
                        nc.tensor.matmul(
                            pz, lhsT=w1sb[:, 0, :], rhs=gt[:, 0, :],
                            start=False, stop=False,
                        )
                        nc.tensor.matmul(
                            pz, lhsT=w1sb[:, 1, :], rhs=gt[:, 1, :],
                            start=False, stop=True,
                        )
                        nc.scalar.activation(out=dst[0:NS, :], in_=pz, func=Copy)
                        if k % S == S - 1:
                            # batched obs of slots 0..S-1 (= z_{t+1} for the
                            # last S steps, slot-major == t ascending)
                            px = xpsum.tile(
                                [S * bg, NS], f32, tag=f"xp{g}", name=f"px{g}"
                            )
                            nc.tensor.matmul(
                                px, lhsT=zhg, rhs=obsb,
                                start=True, stop=True,
                            )
                            nc.scalar.activation(
                                out=xst[g][:, k // S, :], in_=px, func=Copy
                            )
                for g in range(g_groups):
                    dest_t = ds(1, u) if timing_mode else ds(it + 1, u)
                    dest = x[g * bg:(g + 1) * bg, dest_t, :].rearrange(
                        "b (c s) n -> s b c n", s=S
                    )
                    nc.sync.dma_start(out=dest, in_=xst[g])

    nc.compile()
    return nc


def _build_program_unrolled(nt, g_groups=2, dma_chunk=64, obs_chunk=8,
                            timing_mode=False, nt_steps=None, no_obs=False):
    """Fully unrolled time loop: every DMA destination is static, avoiding the
    ~50MB/s dynamic-DMA ucode path; no For_i back-edge barriers."""
    import concourse.bacc as bacc
    import concourse.mybir as mybir
    import concourse.tile as tile
    from contextlib import ExitStack

    f32 = mybir.dt.float32
    bg = B // g_groups
    if nt_steps is None:
        nt_steps = nt
    assert nt % dma_chunk == 0 and dma_chunk % obs_chunk == 0

    nc = bacc.Bacc("TRN2", target_bir_lowering=False, debug=False)

    z0t = nc.dram_tensor("z0t", [NS + 1, B], f32, kind="ExternalInput")
    w2h = nc.dram_tensor("w2h", [NS, NH], f32, kind="ExternalInput")
    w1h = nc.dram_tensor("w1h", [NH, NS], f32, kind="ExternalInput")
    azm = nc.dram_tensor("azm", [NS + 1, NS], f32, kind="ExternalInput")
    obb = nc.dram_tensor("obb", [NS + 1, NS], f32, kind="ExternalInput")
    x_t_size = (dma_chunk + 1) if timing_mode else (nt + 1)
    x = nc.dram_tensor("x", [B, x_t_size, NS], f32, kind="ExternalOutput")

    Copy = mybir.ActivationFunctionType.Copy
    AOp = mybir.AluOpType

    with tile.TileContext(nc) as tc:
        with ExitStack() as ctx:
            const = ctx.enter_context(tc.tile_pool(name="const", bufs=1))
            state = ctx.enter_context(tc.tile_pool(name="state", bufs=1))
            gpool = ctx.enter_context(tc.tile_pool(name="gp", bufs=1))
            xsb = ctx.enter_context(tc.tile_pool(name="xsb", bufs=2))
            ypsum = ctx.enter_context(tc.tile_pool(name="yps", bufs=1, space="PSUM"))
            zpsum = ctx.enter_context(tc.tile_pool(name="zps", bufs=1, space="PSUM"))
            xpsum = ctx.enter_context(tc.tile_pool(name="xps", bufs=2, space="PSUM"))

            w2sb = const.tile([NS, NH], f32, tag="w2")
            w1sb = const.tile([128, 2, NS], f32, tag="w1")
            azsb = const.tile([NS + 1, NS], f32, tag="az")
            obsb = const.tile([NS + 1, NS], f32, tag="ob")
            nc.sync.dma_start(out=w2sb, in_=w2h[:, :])
            nc.sync.dma_start(out=w1sb, in_=w1h.rearrange("(c p) m -> p c m", c=2))
            nc.sync.dma_start(out=azsb, in_=azm[:, :])
            nc.sync.dma_start(out=obsb, in_=obb[:, :])

            zs = []
            for g in range(g_groups):
                za = state.tile([NS + 1, bg], f32, tag=f"zA{g}", name=f"zA{g}")
                zb = state.tile([NS + 1, bg], f32, tag=f"zB{g}", name=f"zB{g}")
                nc.sync.dma_start(out=za, in_=z0t[:, g * bg:(g + 1) * bg])
                nc.vector.memset(zb[NS:NS + 1, :], 1.0)
                zs.append((za, zb))

            for g in range(g_groups):
                px0 = xpsum.tile([bg, 1, NS], f32, tag=f"xp{g}", name=f"px0{g}")
                nc.tensor.matmul(
                    px0[:, 0, :], lhsT=zs[g][0], rhs=obsb, start=True, stop=True
                )
                x0sb = xsb.tile([bg, 1, NS], f32, tag=f"x0{g}", name=f"x0sb{g}")
                nc.scalar.activation(out=x0sb, in_=px0, func=Copy)
                nc.sync.dma_start(out=x[g * bg:(g + 1) * bg, 0:1, :], in_=x0sb)

            xts = [None] * g_groups
            pxs = [None] * g_groups
            for t in range(nt_steps):
                kd = t % dma_chunk
                for g in range(g_groups):
                    za, zb = zs[g]
                    src, dst = (za, zb) if t % 2 == 0 else (zb, za)
                    if kd == 0:
                        xts[g] = xsb.tile(
                            [bg, dma_chunk, NS], f32, tag=f"xt{g}", name=f"xt{g}"
                        )
                    py = ypsum.tile([128, 2, bg], f32, tag=f"y{g}", name=f"py{g}")
                    nc.tensor.matmul(
                        py[:, 0, :], lhsT=w2sb[:, 0:128], rhs=src[0:NS, :],
                        start=True, stop=True,
                    )
                    nc.tensor.matmul(
                        py[:, 1, :], lhsT=w2sb[:, 128:256], rhs=src[0:NS, :],
                        start=True, stop=True,
                    )
                    gt = gpool.tile([128, 2, bg], f32, tag=f"g{g}", name=f"gt{g}")
                    nc.vector.tensor_scalar(
                        out=gt, in0=py, scalar1=-1.0, scalar2=0.0,
                        op0=AOp.max, op1=AOp.min,
                    )
                    pz = zpsum.tile([NS, bg], f32, tag=f"z{g}", name=f"pz{g}")
                    nc.tensor.matmul(pz, lhsT=azsb, rhs=src, start=True, stop=False)
                    nc.tensor.matmul(
                        pz, lhsT=w1sb[:, 0, :], rhs=gt[:, 0, :],
                        start=False, stop=False,
                    )
                    nc.tensor.matmul(
                        pz, lhsT=w1sb[:, 1, :], rhs=gt[:, 1, :],
                        start=False, stop=True,
                    )
                    nc.scalar.activation(out=dst[0:NS, :], in_=pz, func=Copy)
                    if no_obs:
                        continue
                    if t % obs_chunk == 0:
                        pxs[g] = xpsum.tile(
                            [bg, obs_chunk, NS], f32, tag=f"xp{g}", name=f"px{g}"
                        )
                    nc.tensor.matmul(
                        pxs[g][:, t % obs_chunk, :], lhsT=dst, rhs=obsb,
                        start=True, stop=True,
                    )
                    if t % obs_chunk == obs_chunk - 1:
                        nc.scalar.activation(
                            out=xts[g][:, kd - (obs_chunk - 1):kd + 1, :],
                            in_=pxs[g], func=Copy,
                        )
                    if kd == dma_chunk - 1:
                        t0 = 0 if timing_mode else t - (dma_chunk - 1)
                        nc.sync.dma_start(
                            out=x[g * bg:(g + 1) * bg, t0 + 1:t0 + 1 + dma_chunk, :],
                            in_=xts[g],
                        )

    nc.compile()
    return nc


U2 = 128   # v2 time steps per For_i iteration
S2 = 4     # v2 obs history slots (S2*B = 128 partitions for obs matmul)
AZ_MODE = "matmul"  # how A*z+h1p enters the state update (see _build_program_v2)


def _build_program_v2(nt, u, nt_run=None, timing_mode=False, az_mode=None,
                      sched_tweaks=False, split_clamp=False, obs_shift=True):
    """v2: G=1 (all 32 lanes in one chain), bf16 for the mm1 / g@W1h / obs
    matmul paths (both operands bf16), fp32 state carry for the A*z matmul.
    Per step:
      py  = w2b^T zh[prev]          (2 bf16 matmuls, psum fp32)
      g   = clamp(py, -1, 0)        (1 DVE op, bf16 out)
      pz  = azm^T z32 + w1b^T g     (1 fp32 + 2 bf16 matmuls, one psum group)
      zh[slot] = bf16(pz)  [DVE]    z32 = fp32(pz)  [Act]
    obs: every S2 steps one [65,128]x[65,64] bf16 matmul over the zh history,
    one Act copy to the x staging tile; DMA per u-chunk."""
    import concourse.bacc as bacc
    import concourse.mybir as mybir
    import concourse.tile as tile
    from concourse.bass import ds
    from contextlib import ExitStack

    f32 = mybir.dt.float32
    bf16 = mybir.dt.bfloat16
    if nt_run is None:
        nt_run = nt
    if az_mode is None:
        az_mode = AZ_MODE
    S = S2
    assert nt % u == 0 and nt_run % u == 0 and u % S == 0

    nc = bacc.Bacc("TRN2", target_bir_lowering=False, debug=False)

    bq = BS // NCORES  # 32 lanes
    z0t = nc.dram_tensor("z0t", [NS + 1, bq], f32, kind="ExternalInput")
    w2b_d = nc.dram_tensor("w2b", [NS, NH], bf16, kind="ExternalInput")
    w1b_d = nc.dram_tensor("w1b", [NH, NS], bf16, kind="ExternalInput")
    azm_d = nc.dram_tensor("azm", [NS + 1, NS], f32, kind="ExternalInput")
    avec_d = nc.dram_tensor("avec", [NS, 2], f32, kind="ExternalInput")
    obb_d = nc.dram_tensor("obb", [NS + 1, NS], bf16, kind="ExternalInput")
    x_t_size = (u + 1) if timing_mode else (nt + 1)
    x = nc.dram_tensor("x", [bq, x_t_size, NS], f32, kind="ExternalOutput")

    Copy = mybir.ActivationFunctionType.Copy
    AOp = mybir.AluOpType

    with tile.TileContext(nc) as tc:
        with ExitStack() as ctx:
            nb = 2
            const = ctx.enter_context(tc.tile_pool(name="const", bufs=1))
            state = ctx.enter_context(tc.tile_pool(name="state", bufs=1))
            gpool = ctx.enter_context(
                tc.tile_pool(name="gp", bufs=2 if sched_tweaks else 1))
            xsb = ctx.enter_context(tc.tile_pool(name="xsb", bufs=2))
            ypsum = ctx.enter_context(tc.tile_pool(name="yps", bufs=nb, space="PSUM"))
            zpsum = ctx.enter_context(tc.tile_pool(name="zps", bufs=nb, space="PSUM"))
            xpsum = ctx.enter_context(
                tc.tile_pool(name="xps", bufs=1 if split_clamp else 2,
                             space="PSUM"))

            w2sb = const.tile([NS, 2, 128], bf16, tag="w2")
            w1sb = const.tile([128, 2, NS], bf16, tag="w1")
            azsb = const.tile([NS + 1, NS], f32, tag="az")
            avsb = const.tile([NS, 2], f32, tag="av")
            obsb = const.tile([NS + 1, NS], bf16, tag="ob")
            nc.sync.dma_start(out=w2sb, in_=w2b_d.rearrange("k (c m) -> k c m", c=2))
            nc.sync.dma_start(out=w1sb, in_=w1b_d.rearrange("(c p) m -> p c m", c=2))
            nc.sync.dma_start(out=azsb, in_=azm_d[:, :])
            nc.sync.dma_start(out=avsb, in_=avec_d[:, :])
            nc.sync.dma_start(out=obsb, in_=obb_d[:, :])
            a_ap, h1p_ap = avsb[:, 0:1], avsb[:, 1:2]
            Ident = mybir.ActivationFunctionType.Identity

            ewise = az_mode in ("ewise", "ewise_gp")
            # fp32 state for the A*z matmul path (az_mode="matmul" only);
            # ones row comes in via z0t
            z32 = state.tile([NS + 1, bq], f32, tag="z32", name="z32")
            nc.sync.dma_start(out=z32, in_=z0t[:, :])
            # azs parity tiles: azs[k%2] holds A*z_k + h1p (az_mode="ewise")
            azs = [state.tile([NS, bq], f32, tag=f"azs{p}", name=f"azs{p}")
                   for p in range(2)]
            if ewise:
                nc.scalar.activation(out=azs[0], in_=z32[0:NS, :], func=Ident,
                                     scale=a_ap, bias=h1p_ap)
            # bf16 state history [65, S, bq]; slot S-1 starts as bf16(z0)
            zh = state.tile([NS + 1, S, bq], bf16, tag="zh", name="zh")
            nc.vector.memset(zh[NS:NS + 1, :, :], 1.0)
            nc.vector.tensor_scalar(out=zh[0:NS, S - 1, :], in0=z32[0:NS, :],
                                    scalar1=0.0, scalar2=0.0,
                                    op0=AOp.add, op1=AOp.add)

            # t = 0 observation from slot S-1
            px0 = xpsum.tile([bq, NS], f32, tag="xp0", name="px0")
            nc.tensor.matmul(px0, lhsT=zh[:, S - 1, :], rhs=obsb,
                             start=True, stop=True)
            x0sb = xsb.tile([bq, NS], f32, tag="x0", name="x0sb")
            nc.scalar.activation(out=x0sb, in_=px0, func=Ident)
            nc.sync.dma_start(out=x[:, 0, :], in_=x0sb)

            loop_kw = {}
            if sched_tweaks:
                loop_kw = dict(hint_engines=(mybir.EngineType.Pool,),
                               staggered_reset=True)
            else:
                loop_kw = dict(hint_engines=(mybir.EngineType.PE,))
            with tc.For_i(0, nt_run, u, **loop_kw) as it:
                xst = xsb.tile([S * bq, u // S, NS], f32, tag="xt", name="xst")

                def emit_obs(j):
                    px = xpsum.tile([S * bq, NS], f32, tag="xp", name="px")
                    nc.tensor.matmul(px, lhsT=zh, rhs=obsb,
                                     start=True, stop=True)
                    nc.scalar.activation(out=xst[:, j // S, :], in_=px,
                                         func=Ident)

                pending_obs = None
                for k in range(u):
                    slot, prev = k % S, (k - 1) % S
                    if split_clamp:
                        # separate psum banks + g tiles per hidden chunk so
                        # clamp of chunk 0 starts as soon as mm1a lands
                        py0 = ypsum.tile([128, bq], f32, tag="y0")
                        py1 = ypsum.tile([128, bq], f32, tag="y1")
                        nc.tensor.matmul(py0, lhsT=w2sb[:, 0, :],
                                         rhs=zh[0:NS, prev, :],
                                         start=True, stop=True)
                        nc.tensor.matmul(py1, lhsT=w2sb[:, 1, :],
                                         rhs=zh[0:NS, prev, :],
                                         start=True, stop=True)
                        gt0 = gpool.tile([128, bq], bf16, tag="g0")
                        gt1 = gpool.tile([128, bq], bf16, tag="g1")
                        nc.vector.tensor_scalar(out=gt0, in0=py0,
                                                scalar1=-1.0, scalar2=0.0,
                                                op0=AOp.max, op1=AOp.min)
                        nc.vector.tensor_scalar(out=gt1, in0=py1,
                                                scalar1=-1.0, scalar2=0.0,
                                                op0=AOp.max, op1=AOp.min)
                        gta, gtb = gt0, gt1
                    else:
                        py = ypsum.tile([128, 2, bq], f32, tag="y")
                        nc.tensor.matmul(py[:, 0, :], lhsT=w2sb[:, 0, :],
                                         rhs=zh[0:NS, prev, :],
                                         start=True, stop=True)
                        nc.tensor.matmul(py[:, 1, :], lhsT=w2sb[:, 1, :],
                                         rhs=zh[0:NS, prev, :],
                                         start=True, stop=True)
                        gt = gpool.tile([128, 2, bq], bf16, tag="g")
                        nc.vector.tensor_scalar(out=gt, in0=py,
                                                scalar1=-1.0, scalar2=0.0,
                                                op0=AOp.max, op1=AOp.min)
                        gta, gtb = gt[:, 0, :], gt[:, 1, :]
                    if pending_obs is not None:
                        emit_obs(pending_obs)
                        pending_obs = None
                    pz = zpsum.tile([NS, bq], f32, tag="z")
                    if ewise:
                        nc.tensor.matmul(pz, lhsT=w1sb[:, 0, :],
                                         rhs=gta,
                                         start=True, stop=False)
                        nc.tensor.matmul(pz, lhsT=w1sb[:, 1, :],
                                         rhs=gtb,
                                         start=False, stop=True)
                        # z_{k+1} = pz + azs_k, written bf16 into zh (chain)
                        # and fp32 into z32 (feeds the azs recurrence on Act)
                        nc.vector.tensor_tensor(out=zh[0:NS, slot, :],
                                                in0=pz, in1=azs[k % 2],
                                                op=AOp.add)
                        z32eng = (nc.gpsimd if az_mode == "ewise_gp"
                                  else nc.vector)
                        z32eng.tensor_tensor(out=z32[0:NS, :],
                                             in0=pz, in1=azs[k % 2],
                                             op=AOp.add)
                        nc.scalar.activation(out=azs[(k + 1) % 2],
                                             in_=z32[0:NS, :],
                                             func=Ident, scale=a_ap,
                                             bias=h1p_ap)
                    else:
                        nc.tensor.matmul(pz, lhsT=azsb, rhs=z32,
                                         start=True, stop=False)
                        nc.tensor.matmul(pz, lhsT=w1sb[:, 0, :],
                                         rhs=gta,
                                         start=False, stop=False)
                        nc.tensor.matmul(pz, lhsT=w1sb[:, 1, :],
                                         rhs=gtb,
                                         start=False, stop=True)
                        nc.vector.tensor_scalar(out=zh[0:NS, slot, :], in0=pz,
                                                scalar1=0.0, scalar2=0.0,
                                                op0=AOp.add, op1=AOp.add)
                        nc.scalar.activation(out=z32[0:NS, :], in_=pz,
                                             func=Ident)
                    if k % S == S - 1:
                        if obs_shift and k < u - 1:
                            pending_obs = k
                        else:
                            emit_obs(k)
                for s in range(S):
                    dest_t = (ds(1 + s, u // S, S) if timing_mode
                              else ds(it + 1 + s, u // S, S))
                    nc.sync.dma_start(out=x[:, dest_t, :],
                                      in_=xst[s * bq:(s + 1) * bq, :, :])

    nc.compile()
    return nc


U3 = 64    # v3 time steps per For_i iteration
S3 = 4     # v3 obs history slots (S3*B = 128 partitions for obs matmul)
STAGGER3 = False  # staggered_reset on the v3 For_i (measured slower when on)


def _build_program_v3(nt, u=None, nt_run=None, timing_mode=False):
    """v3: depth-2 unrolled y-recurrence.  The carried hidden pre-activation
    y_t = W2h z_t lives in PSUM; per step only clamp + 4 M0 matmuls are on the
    serial critical path:

      g_k     = clamp(y_k, -1, 0)                       (1 DVE op from PSUM)
      y_{k+1} = Wa2 zh_{k-1} + M1 g_{k-1} + M0 g_k + c2 (10 bf16 MMs, PSUM acc;
                 only the 4 M0 g_k MMs wait on the clamp)
      pz      = [diag(A);h1p] z32 + W1h g_k = z_{k+1}   (fp32 state path)
      zh_k    <- bf16(pz_k)  [DVE]   z32_k <- fp32(pz_k) [Act]  (deferred one
                 step so the clamp stays at the head of both engine queues)

    where Wa2 = diag(A^2) W2h (+bias row (1+A)h1p @ W2h), M0 = W1h W2h,
    M1 = W1h diag(A) W2h.  obs every S3 steps from the bf16 zh ring, exactly
    as v2.  First u steps are peeled (step 0 uses the depth-1 form); the
    For_i loop then runs steps u..nt_run.
    """
    import concourse.bacc as bacc
    import concourse.mybir as mybir
    import concourse.tile as tile
    from concourse.bass import ds
    from contextlib import ExitStack

    f32 = mybir.dt.float32
    bf16 = mybir.dt.bfloat16
    if u is None:
        u = U3
    if nt_run is None:
        nt_run = nt
    S = S3
    assert nt % u == 0 and nt_run % u == 0 and u % S == 0 and nt_run >= 2 * u

    nc = bacc.Bacc("TRN2", target_bir_lowering=False, debug=False)

    bq = B
    z0t = nc.dram_tensor("z0t", [NS + 1, bq], f32, kind="ExternalInput")
    w2f_d = nc.dram_tensor("w2f", [NS, 2, 128], f32, kind="ExternalInput")
    wab_d = nc.dram_tensor("wab", [NS + 1, 2, 128], bf16, kind="ExternalInput")
    wa2b_d = nc.dram_tensor("wa2b", [NS + 1, 2, 128], bf16, kind="ExternalInput")
    m0b_d = nc.dram_tensor("m0b", [128, 2, 2, 128], bf16, kind="ExternalInput")
    m1b_d = nc.dram_tensor("m1b", [128, 2, 2, 128], bf16, kind="ExternalInput")
    w1b_d = nc.dram_tensor("w1b", [128, 2, NS], bf16, kind="ExternalInput")
    azm_d = nc.dram_tensor("azm", [NS + 1, NS], f32, kind="ExternalInput")
    obb_d = nc.dram_tensor("obb", [NS + 1, NS], bf16, kind="ExternalInput")
    x_t_size = (u + 1) if timing_mode else (nt + 1)
    x = nc.dram_tensor("x", [bq, x_t_size, NS], f32, kind="ExternalOutput")

    Ident = mybir.ActivationFunctionType.Identity
    AOp = mybir.AluOpType

    with tile.TileContext(nc) as tc:
        with ExitStack() as ctx:
            const = ctx.enter_context(tc.tile_pool(name="const", bufs=1))
            state = ctx.enter_context(tc.tile_pool(name="state", bufs=1))
            gpool = ctx.enter_context(tc.tile_pool(name="gp", bufs=1))
            xsb = ctx.enter_context(tc.tile_pool(name="xsb", bufs=2))
            ypsum = ctx.enter_context(tc.tile_pool(name="yps", bufs=1, space="PSUM"))
            zpsum = ctx.enter_context(tc.tile_pool(name="zps", bufs=1, space="PSUM"))
            xpsum = ctx.enter_context(tc.tile_pool(name="xps", bufs=2, space="PSUM"))

            w2f = const.tile([NS, 2, 128], f32, tag="w2f")
            wasb = const.tile([NS + 1, 2, 128], bf16, tag="wa")
            wa2sb = const.tile([NS + 1, 2, 128], bf16, tag="wa2")
            m0sb = const.tile([128, 2, 2, 128], bf16, tag="m0")
            m1sb = const.tile([128, 2, 2, 128], bf16, tag="m1")
            w1sb = const.tile([128, 2, NS], bf16, tag="w1")
            azsb = const.tile([NS + 1, NS], f32, tag="az")
            obsb = const.tile([NS + 1, NS], bf16, tag="ob")
            nc.sync.dma_start(out=w2f, in_=w2f_d[:, :, :])
            nc.sync.dma_start(out=wasb, in_=wab_d[:, :, :])
            nc.sync.dma_start(out=wa2sb, in_=wa2b_d[:, :, :])
            nc.sync.dma_start(out=m0sb, in_=m0b_d[:, :, :, :])
            nc.sync.dma_start(out=m1sb, in_=m1b_d[:, :, :, :])
            nc.sync.dma_start(out=w1sb, in_=w1b_d[:, :, :])
            nc.sync.dma_start(out=azsb, in_=azm_d[:, :])
            nc.sync.dma_start(out=obsb, in_=obb_d[:, :])

            # fp32 state z32 [65, bq] (ones row via z0t), bf16 ring zh
            z32 = state.tile([NS + 1, bq], f32, tag="z32", name="z32")
            nc.sync.dma_start(out=z32, in_=z0t[:, :])
            zh = state.tile([NS + 1, S, bq], bf16, tag="zh", name="zh")
            nc.vector.memset(zh[NS:NS + 1, :, :], 1.0)
            # zh slot 0 <- z_0
            nc.vector.tensor_scalar(out=zh[0:NS, 0, :], in0=z32[0:NS, :],
                                    scalar1=0.0, scalar2=0.0,
                                    op0=AOp.add, op1=AOp.add)

            # fixed ping-pong tiles (loop-carried across For_i iterations;
            # pool auto-rotation cannot cross the loop boundary).
            # y tiles are padded to 512 elems/chunk so each of the two
            # 128-row output chunks owns a full PSUM bank: accumulation
            # groups for the two chunks interleave, and start_tensor_calc
            # zeroes a whole bank.
            pys = [ypsum.tile([128, 2, 512], f32, tag=f"y{p}", name=f"py{p}")
                   for p in range(2)]
            pzs = [zpsum.tile([NS, bq], f32, tag=f"z{p}", name=f"pz{p}")
                   for p in range(2)]
            gts = [gpool.tile([128, 2, bq], bf16, tag=f"g{p}", name=f"gt{p}")
                   for p in range(2)]

            # bootstrap y_0 = W2h z_0 (fp32, one-time) into pys[0]
            for j in range(2):
                nc.tensor.matmul(pys[0][:, j, 0:bq], lhsT=w2f[:, j, :],
                                 rhs=z32[0:NS, :], start=True, stop=True)

            def emit_step(k, first=False):
                """One scan step.  k = local step index; parity/slot follow
                the global step because u % 2 == 0 and u % S == 0.
                first=True -> global step 0 (depth-1 y form, no zh/z32
                writeback since z_0 came from DRAM)."""
                slot = k % S
                p = k % 2
                py_cur = pys[p]
                pz_prev = pzs[p]          # holds z_k (written at step k-1)
                gt_prev = gts[(k - 1) % 2]

                # 1. g_k = clamp(y_k) — keep at the head of the DVE queue
                gt = gts[p]
                nc.vector.tensor_scalar(out=gt, in0=py_cur[:, :, 0:bq],
                                        scalar1=-1.0, scalar2=0.0,
                                        op0=AOp.max, op1=AOp.min)

                # 2. deferred state writeback of z_k (reads pz from step k-1)
                if not first:
                    nc.vector.tensor_scalar(out=zh[0:NS, slot, :], in0=pz_prev,
                                            scalar1=0.0, scalar2=0.0,
                                            op0=AOp.add, op1=AOp.add)
                    nc.scalar.activation(out=z32[0:NS, :], in_=pz_prev,
                                         func=Ident)

                # 3. y_{k+1} into the other PSUM bank.
                # depth-1: y_{k+1} = M0 g_k + Wa zh(z_k) + c.  The PE is the
                # saturated resource (~87ns/MM incl LDWEIGHTS), so fewer MMs
                # beats the shorter dependency chain of the depth-2 form.
                # M0 first (waits on the clamp), Wa last (zh(z_k) is written
                # by this step's deferred DVE writeback, ready later anyway).
                py_new = pys[1 - p]
                for ki in range(2):
                    for j in range(2):
                        nc.tensor.matmul(py_new[:, j, 0:bq],
                                         lhsT=m0sb[:, ki, j, :],
                                         rhs=gt[:, ki, :],
                                         start=(ki == 0), stop=False)
                for j in range(2):
                    nc.tensor.matmul(py_new[:, j, 0:bq], lhsT=wasb[:, j, :],
                                     rhs=zh[:, slot, :], start=False,
                                     stop=True)

                # 4. obs of the completed ring window (before az/W1h so the
                # next step's zh write isn't stalled by the WAR on slot 0)
                ox = None
                if slot == S - 1:
                    px = xpsum.tile([S * bq, NS], f32, tag="xp", name="px")
                    nc.tensor.matmul(px, lhsT=zh, rhs=obsb,
                                     start=True, stop=True)
                    ox = px

                # 5. fp32 z recurrence: pz = z_{k+1}
                pz = pzs[1 - p]
                nc.tensor.matmul(pz, lhsT=azsb, rhs=z32, start=True, stop=False)
                for ki in range(2):
                    nc.tensor.matmul(pz, lhsT=w1sb[:, ki, :], rhs=gt[:, ki, :],
                                     start=False, stop=(ki == 1))
                return ox

            def emit_obs_copy(px, xst, c):
                nc.scalar.activation(out=xst[:, c, :], in_=px, func=Ident)

            # ---- peeled first chunk (steps 0..u-1), static DMA dests ----
            xst = xsb.tile([S * bq, u // S, NS], f32, tag="xt", name="xst")
            for k in range(u):
                px = emit_step(k, first=(k == 0))
                if px is not None:
                    emit_obs_copy(px, xst, k // S)
            for s in range(S):
                dest_t = ds(s, u // S, S)
                nc.sync.dma_start(out=x[:, dest_t, :],
                                  in_=xst[s * bq:(s + 1) * bq, :, :])

            # ---- main loop: steps u..nt_run ----
            with tc.For_i(u, nt_run, u,
                          hint_engines=(mybir.EngineType.PE,),
                          staggered_reset=STAGGER3) as it:
                xst = xsb.tile([S * bq, u // S, NS], f32, tag="xt", name="xstL")
                for k in range(u):
                    px = emit_step(k)
                    if px is not None:
                        emit_obs_copy(px, xst, k // S)
                for s in range(S):
                    dest_t = (ds(s, u // S, S) if timing_mode
                              else ds(it + s, u // S, S))
                    nc.sync.dma_start(out=x[:, dest_t, :],
                                      in_=xst[s * bq:(s + 1) * bq, :, :])

            # ---- epilogue: x_nt from the final pz (step nt-1 wrote parity
            # (nt)%2 = 0 since it produced z_nt into pzs[1 - (nt-1)%2]) ----
            nc.vector.tensor_scalar(out=zh[0:NS, 0, :],
                                    in0=pzs[nt_run % 2],
                                    scalar1=0.0, scalar2=0.0,
                                    op0=AOp.add, op1=AOp.add)
            pxf = xpsum.tile([S * bq, NS], f32, tag="xp", name="pxf")
            nc.tensor.matmul(pxf[0:bq, :], lhsT=zh[:, 0, :], rhs=obsb,
                             start=True, stop=True)
            xf = xsb.tile([bq, NS], f32, tag="xf", name="xf")
            nc.scalar.activation(out=xf, in_=pxf[0:bq, :], func=Ident)
            t_last = u if timing_mode else nt
            nc.sync.dma_start(out=x[:, t_last, :], in_=xf)

    nc.compile()
    return nc


def _host_precompute_v3(z0, A, W1, W2, h1, h2, OB, Ob):
    """Constants for _build_program_v3, in final on-device lhsT layouts."""
    import ml_dtypes

    bf16 = ml_dtypes.bfloat16
    A64 = A.astype(np.float64)
    h2d = h2.astype(np.float64)
    sigma = np.where(h2d >= 0, 1.0, -1.0)
    absh = np.maximum(np.abs(h2d), 1e-30)
    w2h = W2.astype(np.float64) * (sigma / absh)[None, :]        # [64, 256]
    w1h = W1.astype(np.float64) * absh[:, None]                  # [256, 64]
    h1p = h1.astype(np.float64) + np.maximum(h2d, 0) @ W1.astype(np.float64)

    def aug(mat, row):  # [64, 256] + bias row -> [65, 2, 128]
        full = np.concatenate([mat, row[None, :]], 0)
        return full.reshape(NS + 1, 2, 128)

    wa = aug(A64[:, None] * w2h, h1p @ w2h)
    wa2 = aug((A64 ** 2)[:, None] * w2h, ((1.0 + A64) * h1p) @ w2h)
    m0 = w1h @ w2h                                               # [256, 256]
    m1 = (w1h * A64[None, :]) @ w2h

    def blk(m):  # [256, 256] -> [128, ki, kj, 128]
        return np.ascontiguousarray(
            m.reshape(2, 128, 2, 128).transpose(1, 0, 2, 3))

    w1b = np.ascontiguousarray(
        w1h.reshape(2, 128, NS).transpose(1, 0, 2))              # [128, 2, 64]
    azm = np.zeros([NS + 1, NS], np.float64)
    azm[np.arange(NS), np.arange(NS)] = A64
    azm[NS] = h1p
    obb = np.concatenate([OB.astype(np.float64),
                          Ob.astype(np.float64)[None, :]], 0)
    w2f = np.ascontiguousarray(
        w2h.reshape(NS, 2, 128))                                 # [64, 2, 128]
    return {
        "w2f": w2f.astype(np.float32),
        "wab": wa.astype(np.float32).astype(bf16),
        "wa2b": wa2.astype(np.float32).astype(bf16),
        "m0b": blk(m0).astype(np.float32).astype(bf16),
        "m1b": blk(m1).astype(np.float32).astype(bf16),
        "w1b": w1b.astype(np.float32).astype(bf16),
        "azm": azm.astype(np.float32),
        "obb": obb.astype(np.float32).astype(bf16),
    }


_prog_cache = {}


def _get_program(nt, u, g_groups):
    key = (nt, u, g_groups)
    if key not in _prog_cache:
        _prog_cache[key] = _build_program(nt, u, g_groups)
    return _prog_cache[key]


def make_runner(nc, n_cores=NCORES):
    """Multi-core PJRT runner (mirrors bass2jax.run_bass_via_pjrt), with a
    unique jit body name per program: the neuron NEFF disk cache keys on the
    module file_prefix, which ignores the embedded BIR — identical I/O
    signatures would otherwise collide across different programs."""
    import uuid

    import jax
    import concourse.mybir as mybir
    from concourse import bass2jax
    from concourse.bass2jax import _bass_exec_p, partition_id_tensor
    from jax.sharding import Mesh, PartitionSpec
    from jax.experimental.shard_map import shard_map

    bass2jax.install_neuronx_cc_hook()
    partition_name = nc.partition_id_tensor.name if nc.partition_id_tensor else None
    in_names, out_names, out_avals, zero_outs = [], [], [], []
    for alloc in nc.m.functions[0].allocations:
        if not isinstance(alloc, mybir.MemoryLocationSet):
            continue
        name = alloc.memorylocations[0].name
        if alloc.kind == "ExternalInput":
            if name != partition_name:
                in_names.append(name)
        elif alloc.kind == "ExternalOutput":
            shape = tuple(alloc.tensor_shape)
            dtype = mybir.dt.np(alloc.dtype)
            out_names.append(name)
            out_avals.append(jax.core.ShapedArray(shape, dtype))
            zero_outs.append(np.zeros(shape, dtype))
    n_params = len(in_names)
    n_outs = len(out_avals)
    in_names_all = in_names + out_names + ([partition_name] if partition_name else [])
    donate = tuple(range(n_params, n_params + n_outs))

    def _body(*args):
        operands = list(args)
        if partition_name is not None:
            operands.append(partition_id_tensor())
        outs = _bass_exec_p.bind(
            *operands,
            out_avals=tuple(out_avals),
            in_names=tuple(in_names_all),
            out_names=tuple(out_names),
            lowering_input_output_aliases=(),
            sim_require_finite=True,
            sim_require_nnan=True,
            nc=nc,
        )
        return tuple(outs)

    _body.__name__ = f"body_{uuid.uuid4().hex[:12]}"

    devices = jax.devices()[:n_cores]
    assert len(devices) == n_cores
    mesh = Mesh(np.asarray(devices), ("core",))
    sharded = jax.jit(
        shard_map(
            _body, mesh=mesh,
            in_specs=(PartitionSpec("core"),) * (n_params + n_outs),
            out_specs=(PartitionSpec("core"),) * n_outs,
            check_rep=False,
        ),
        donate_argnums=donate,
        keep_unused=True,
    )

    def run(in_maps):
        import time as _time

        per_core = [[np.asarray(m[n]) for n in in_names] for m in in_maps]
        concat_in = [
            np.concatenate([per_core[c][i] for c in range(n_cores)], 0)
            for i in range(n_params)
        ]
        concat_zeros = [
            np.zeros((n_cores * z.shape[0], *z.shape[1:]), z.dtype)
            for z in zero_outs
        ]
        t0 = _time.time()
        out = sharded(*concat_in, *concat_zeros)
        out = [np.asarray(o) for o in out]
        dt = _time.time() - t0
        res = [
            {
                n: out[i].reshape(n_cores, *out_avals[i].shape)[c]
                for i, n in enumerate(out_names)
            }
            for c in range(n_cores)
        ]
        return res, dt

    return run


def _host_precompute(z0, A, W1, W2, h1, h2, OB, Ob):
    h2d = h2.astype(np.float64)
    sigma = np.where(h2d >= 0, 1.0, -1.0)
    absh = np.maximum(np.abs(h2d), 1e-30)
    w2h = (W2.astype(np.float64) * (sigma / absh)[None, :]).astype(np.float32)
    w1h = (W1.astype(np.float64) * absh[:, None]).astype(np.float32)
    h1p = (h1.astype(np.float64) + np.maximum(h2d, 0) @ W1.astype(np.float64)).astype(
        np.float32
    )
    azm = np.zeros([NS + 1, NS], np.float32)
    azm[np.arange(NS), np.arange(NS)] = A.astype(np.float32)
    azm[NS] = h1p
    obb = np.concatenate(
        [OB.astype(np.float32), Ob.astype(np.float32)[None, :]], axis=0
    )
    return w2h, w1h, azm, obb


KVER = "v3"   # which program kernel() runs: "v2" or "v3"


def kernel_v3(z0, A, W1, W2, h1, h2, OB, Ob, nt):
    consts = _host_precompute_v3(z0, A, W1, W2, h1, h2, OB, Ob)
    in_maps = []
    for c in range(NCORES):
        zslice = z0[c * B:(c + 1) * B]  # [B, NS]
        z0t = np.concatenate(
            [zslice.T.copy(), np.ones([1, B], np.float32)], axis=0
        )
        in_maps.append({"z0t": z0t, **consts})

    key = ("v3", nt, U3)
    if key not in _prog_cache:
        nc = _build_program_v3(nt)
        _prog_cache[key] = (nc, make_runner(nc))
    nc, run = _prog_cache[key]
    results, _ = run(in_maps)
    xs = [results[c]["x"] for c in range(NCORES)]
    return np.concatenate(xs, axis=0).astype(np.float32)


def kernel(**inputs):
    import ml_dtypes

    z0 = np.asarray(inputs["z0"], np.float32)
    A = np.asarray(inputs["A"], np.float32)
    W1 = np.asarray(inputs["W1"], np.float32)
    W2 = np.asarray(inputs["W2"], np.float32)
    h1 = np.asarray(inputs["h1"], np.float32)
    h2 = np.asarray(inputs["h2"], np.float32)
    OB = np.asarray(inputs["OB"], np.float32)
    Ob = np.asarray(inputs["Ob"], np.float32)
    nt = int(inputs["nt"])
    assert nt == NT and z0.shape == (BS, NS)

    if KVER == "v3":
        return kernel_v3(z0, A, W1, W2, h1, h2, OB, Ob, nt)

    w2h, w1h, azm, obb = _host_precompute(z0, A, W1, W2, h1, h2, OB, Ob)
    bf16 = ml_dtypes.bfloat16
    w2b = w2h.astype(bf16)
    w1b = w1h.astype(bf16)
    obb16 = obb.astype(bf16)

    in_maps = []
    for c in range(NCORES):
        zslice = z0[c * B:(c + 1) * B]  # [B, NS]
        z0t = np.concatenate(
            [zslice.T.copy(), np.ones([1, B], np.float32)], axis=0
        )  # [NS+1, B]
        avec = np.stack([A.astype(np.float32), azm[NS]], axis=1)  # [64, 2]
        in_maps.append(
            {"z0t": z0t, "w2b": w2b, "w1b": w1b, "azm": azm, "avec": avec,
             "obb": obb16}
        )

    key = ("v2", nt, U2)
    if key not in _prog_cache:
        nc = _build_program_v2(nt, U2)
        _prog_cache[key] = (nc, make_runner(nc))
    nc, run = _prog_cache[key]
    global _last_in_maps, _last_runner
    _last_in_maps, _last_runner = in_maps, run
    results, _ = run(in_maps)
    xs = [results[c]["x"] for c in range(NCORES)]
    return np.concatenate(xs, axis=0).astype(np.float32)


_last_in_maps = None
_last_runner = None

